# revision 1
# baseline (speedup 1.0000x reference)
"""Cross_Atten_Lite_split Trainium2 Bass kernel.

Sharding: 8 cores = (batch b in 0..3) x (query-half qh in 0..1).
Each core computes both attention heads for 2048 queries x 4096 keys of
its batch. No collectives. Math rewrites (validated vs reference):
  - eval-mode BN on x1/x2 folded into kq1_w/kq2_w (+bias).
  - channel_shuffle is a permutation of the shared q/k contraction axis
    -> eliminated;  k_h = [kq1[:,64h:64h+32]; kq2[:,64h:64h+32]],
    q_h likewise from rows 64h+32:64h+64.
  - K bias cancels in softmax (adds a per-query-row constant); dropped.
  - final BN + w_scale folded into out_w/out_b.
  - softmax without max-subtraction (max |score| ~ 67.5 < 88, fp32 safe).
  - softmax denominator via ones-augmented V (row 64 of PV output).
Matmuls run as float32r (fp32 bits, full-speed PE path at N=512).

Built on bacc.Bacc + nc.compile(): generate_event_semaphores splits
multi-wait instructions to satisfy the TRN2 1-wait-per-instruction
constraint.
"""

import numpy as np
from contextlib import ExitStack

import concourse.bass as bass
import concourse.bacc as bacc
import concourse.mybir as mybir
import concourse.tile as tile
from concourse.bass_utils import run_bass_kernel_spmd
from concourse.masks import make_identity

F32 = mybir.dt.float32
F32R = mybir.dt.float32r
AF = mybir.ActivationFunctionType

C = 256          # channels (INC1 == INC2)
N = 4096         # tokens per batch (64*64)
NQ = 2048        # queries per core
NT = 512         # free-dim tile size


def build_bass():
    nc = bacc.Bacc("TRN2", target_bir_lowering=False, debug=False, num_devices=8)

    x1T = nc.dram_tensor("x1T", [C, N], F32R, kind="ExternalInput").ap()
    x2T = nc.dram_tensor("x2T", [C, N], F32R, kind="ExternalInput").ap()
    xT = nc.dram_tensor("xT", [C, N], F32R, kind="ExternalInput").ap()
    wkq1 = nc.dram_tensor("wkq1", [2, 128, 128], F32R, kind="ExternalInput").ap()
    wkq2 = nc.dram_tensor("wkq2", [2, 128, 128], F32R, kind="ExternalInput").ap()
    wv = nc.dram_tensor("wv", [2, 128, 128], F32R, kind="ExternalInput").ap()
    wout = nc.dram_tensor("wout", [128, 256], F32R, kind="ExternalInput").ap()
    bq = nc.dram_tensor("bq", [128, 1], F32, kind="ExternalInput").ap()
    bv = nc.dram_tensor("bv", [128, 1], F32, kind="ExternalInput").ap()
    bout = nc.dram_tensor("bout", [2, 128, 1], F32, kind="ExternalInput").ap()
    onesd = nc.dram_tensor("onesd", [128, 64], F32R, kind="ExternalInput").ap()
    outT = nc.dram_tensor("outT", [C, NQ], F32, kind="ExternalOutput").ap()

    with ExitStack() as ctx:
        tc = ctx.enter_context(tile.TileContext(nc))
        const = ctx.enter_context(tc.tile_pool(name="const", bufs=1))
        pers = ctx.enter_context(tc.tile_pool(name="pers", bufs=1))

        # constants
        w_kq1 = [const.tile([128, 128], F32R, name=f"wkq1_{g}") for g in range(2)]
        w_kq2 = [const.tile([128, 128], F32R, name=f"wkq2_{g}") for g in range(2)]
        w_v = [const.tile([128, 128], F32R, name=f"wv_{g}") for g in range(2)]
        w_out = const.tile([128, 256], F32R, name="wout")
        b_q = const.tile([128, 1], F32, name="bq")
        b_v = const.tile([128, 1], F32, name="bv")
        b_out = [const.tile([128, 1], F32, name=f"bout_{g}") for g in range(2)]
        ident = const.tile([128, 128], F32, name="ident")
        ones1 = const.tile([1, 64], F32R, name="ones1")



        for g in range(2):
            nc.sync.dma_start(out=w_kq1[g][:], in_=wkq1[g])
            nc.sync.dma_start(out=w_kq2[g][:], in_=wkq2[g])
            nc.sync.dma_start(out=w_v[g][:], in_=wv[g])
            nc.sync.dma_start(out=b_out[g][:], in_=bout[g])
        nc.sync.dma_start(out=w_out[:], in_=wout[:])
        nc.sync.dma_start(out=b_q[:], in_=bq[:])
        nc.sync.dma_start(out=b_v[:], in_=bv[:])
        make_identity(nc, ident[:])
        nc.sync.dma_start(out=ones1[:], in_=onesd[0:1, 0:64])

        # persistent SBUF
        KT = pers.tile([128, N], F32R, name="KT")      # rows k1a,k1b,k2a,k2b
        QT = pers.tile([128, NQ], F32R, name="QT")     # rows q1a,q1b,q2a,q2b
        Vtok = pers.tile([128, 32 * 130], F32R, name="Vtok")
        xsb = [pers.tile([128, N], F32R, name=f"xsb_{g}") for g in range(2)]
        x1sb = [pers.tile([128, N], F32R, name=f"x1sb_{g}") for g in range(2)]
        x2sb = [pers.tile([128, N], F32R, name=f"x2sb_{g}") for g in range(2)]
        Ocat = pers.tile([128, NQ], F32R, name="Ocat")

        # fill the two ones-columns of each Vtok m-block via strided DMA
        vtok3 = Vtok.rearrange("p (m c) -> p m c", c=130)
        nc.sync.dma_start(out=vtok3[:, :, 64:65], in_=onesd[:, 0:32].rearrange("p (m c) -> p m c", c=1))
        nc.sync.dma_start(out=vtok3[:, :, 129:130], in_=onesd[:, 32:64].rearrange("p (m c) -> p m c", c=1))
        # DVE pre-touch of bias consts so later DVE ops don't wait on DMA queues
        btch = const.tile([128, 4], F32, name="btch")
        nc.vector.tensor_copy(btch[:, 0:1], b_v[:])
        nc.vector.tensor_copy(btch[:, 1:2], b_q[:])
        nc.vector.tensor_copy(btch[:, 2:3], b_out[0][:])
        nc.vector.tensor_copy(btch[:, 3:4], b_out[1][:])
        for g in range(2):
            for t in range(8):
                cs = slice(t * NT, (t + 1) * NT)
                rs = slice(128 * g, 128 * (g + 1))
                nc.sync.dma_start(out=xsb[g][:, cs], in_=xT[rs, cs])
                nc.sync.dma_start(out=x1sb[g][:, cs], in_=x1T[rs, cs])
                nc.sync.dma_start(out=x2sb[g][:, cs], in_=x2T[rs, cs])

        poolE = ctx.enter_context(tc.tile_pool(name="poolE", bufs=2))
        small = ctx.enter_context(tc.tile_pool(name="small", bufs=2))
        pout = ctx.enter_context(tc.tile_pool(name="pout", bufs=8))
        # ---------- Phase A: projections ----------
        with ExitStack() as actx:
            pvt = actx.enter_context(tc.tile_pool(name="pvt", bufs=2))
            poolA = actx.enter_context(tc.tile_pool(name="poolA", bufs=3, space="PSUM"))
            poolT = actx.enter_context(tc.tile_pool(name="poolT", bufs=4, space="PSUM"))

            for t in range(8):
                cs = slice(t * NT, (t + 1) * NT)
                if t >= 1:
                    # DVE sem is monotone: joining on iter t-1's last DVE
                    # write covers every DVE dep from iters <= t-1
                    mprev = 4 * (t - 1) + 3
                kq1p = poolA.tile([128, NT], F32, tag="mmA", name=f"kq1p_{t}")
                nc.tensor.matmul(kq1p[:], w_kq1[0][:], x1sb[0][:, cs], start=True, stop=False)
                nc.tensor.matmul(kq1p[:], w_kq1[1][:], x1sb[1][:, cs], start=False, stop=True)
                kq2p = poolA.tile([128, NT], F32, tag="mmA", name=f"kq2p_{t}")
                nc.tensor.matmul(kq2p[:], w_kq2[0][:], x2sb[0][:, cs], start=True, stop=False)
                nc.tensor.matmul(kq2p[:], w_kq2[1][:], x2sb[1][:, cs], start=False, stop=True)
                vp = poolA.tile([128, NT], F32, tag="mmA", name=f"vp_{t}")
                nc.tensor.matmul(vp[:], w_v[0][:], xsb[0][:, cs], start=True, stop=False)
                nc.tensor.matmul(vp[:], w_v[1][:], xsb[1][:, cs], start=False, stop=True)

                # scatter K/Q rows straight from PSUM (DVE only -> single sem);
                # Q bias applied during the scatter (tensor_scalar_add)
                nc.vector.tensor_copy(KT[0:32, cs], kq1p[0:32, :])
                nc.vector.tensor_copy(KT[32:64, cs], kq2p[0:32, :])
                nc.vector.tensor_copy(KT[64:96, cs], kq1p[64:96, :])
                nc.vector.tensor_copy(KT[96:128, cs], kq2p[64:96, :])
                if t < 4:  # query half
                    nc.scalar.activation(QT[0:32, cs], kq1p[32:64, :], AF.Identity, bias=b_q[0:32, :])
                    nc.scalar.activation(QT[32:64, cs], kq2p[32:64, :], AF.Identity, bias=b_q[32:64, :])
                    nc.scalar.activation(QT[64:96, cs], kq1p[96:128, :], AF.Identity, bias=b_q[64:96, :])
                    nc.scalar.activation(QT[96:128, cs], kq2p[96:128, :], AF.Identity, bias=b_q[96:128, :])
                VT = pvt.tile([128, NT], F32, tag="VT", name=f"VT_{t}")
                nc.scalar.activation(VT[:], vp[:], AF.Identity, bias=b_v[:])

                # transpose V for PV matmuls: Vtok[m] cols 0:64 = v1, 65:129 = v2
                for s in range(4):
                    m = 4 * t + s
                    ms = slice(s * 128, (s + 1) * 128)
                    tp = poolT.tile([128, 128], F32, tag="tp", name=f"tp_{m}")
                    nc.tensor.transpose(tp[:], VT[:, ms], ident[:])
                    nc.vector.tensor_copy(Vtok[:, m * 130:m * 130 + 64], tp[:, 0:64])
                    nc.vector.tensor_copy(Vtok[:, m * 130 + 65:m * 130 + 129], tp[:, 64:128])


        # ---------- Phase B: attention ----------
        with ExitStack() as bctx:
            poolS = bctx.enter_context(tc.tile_pool(name="poolS", bufs=2, space="PSUM"))
            poolO = bctx.enter_context(tc.tile_pool(name="poolO", bufs=1, space="PSUM"))
            poolCp = bctx.enter_context(tc.tile_pool(name="poolCp", bufs=1, space="PSUM"))

            for h in range(2):
                hs = slice(64 * h, 64 * (h + 1))
                for j in range(4):
                    qs = slice(j * NT, (j + 1) * NT)
                    op = poolO.tile([65, NT], F32, tag="op", name=f"op_{h}_{j}")
                    mstart = 0
                    blk = 0
                    while mstart < 32:
                        mk = min(3, 32 - mstart)
                        sp = poolS.tile([128, 3 * NT], F32, tag="sp", name=f"sp_{h}_{j}_{blk}")
                        for b4 in range(mk):
                            m = mstart + b4
                            nc.tensor.matmul(
                                sp[:, b4 * NT:(b4 + 1) * NT],
                                KT[hs, m * 128:(m + 1) * 128],
                                QT[hs, qs],
                                start=True, stop=True)
                        et = poolE.tile([128, 3 * NT], F32R, tag="et", name=f"et_{h}_{j}_{blk}")
                        nc.scalar.activation(et[:, 0:mk * NT], sp[:, 0:mk * NT],
                                             AF.Exp, scale=0.125)
                        for b4 in range(mk):
                            m = mstart + b4
                            nc.tensor.matmul(
                                op[:],
                                Vtok[:, m * 130 + 65 * h:m * 130 + 65 * h + 65],
                                et[:, b4 * NT:(b4 + 1) * NT],
                                start=(m == 0),
                                stop=(m == 31))
                        mstart += mk
                        blk += 1

                    # normalize: o[d, n] * (1 / denom[n]); denom = op[64, :]
                    rec = small.tile([1, NT], F32R, tag="rec", name=f"rec_{h}_{j}")
                    with nc.allow_low_precision(reason="f32r is fp32 bits"):
                        nc.vector.reciprocal(rec[:], op[64:65, :])
                    rb = poolCp.tile([64, NT], F32, tag="rb", name=f"rb_{h}_{j}")
                    nc.tensor.matmul(rb[:], ones1[:], rec[:], start=True, stop=True)
                    rbs = small.tile([64, NT], F32, tag="rbs", name=f"rbs_{h}_{j}")
                    nc.vector.tensor_copy(rbs[:], rb[:])
                    nc.vector.tensor_mul(Ocat[hs, qs], op[0:64, :], rbs[:])

            # ---------- Phase C: output projection + bias + residual ----------
            for j in range(4):
                qs = slice(j * NT, (j + 1) * NT)
                for g in range(2):
                    pp = poolCp.tile([128, NT], F32, tag="rb", name=f"pp_{j}_{g}")
                    nc.tensor.matmul(pp[:], w_out[:, 128 * g:128 * (g + 1)],
                                     Ocat[:, qs], start=True, stop=True)
                    osb = pout.tile([128, NT], F32, tag="osb", name=f"osb_{j}_{g}")
                    nc.vector.scalar_tensor_tensor(
                        osb[:], pp[:], b_out[g][:], xsb[g][:, qs].bitcast(F32),
                        op0=bass.mybir.AluOpType.add, op1=bass.mybir.AluOpType.add)
                    nc.sync.dma_start(out=outT[128 * g:128 * (g + 1), qs], in_=osb[:])

    nc.compile()
    return nc


_NC = None


def _get_nc():
    global _NC
    if _NC is None:
        _NC = build_bass()
    return _NC


def kernel(**inputs):
    out, _ = _run(inputs, trace=False)
    return out


def _run(inputs, trace=False):
    eps = 1e-5
    f32 = np.float32
    inp = {k: np.asarray(v, dtype=np.float32) for k, v in inputs.items()}

    s1 = inp['bn1_g'] / np.sqrt(inp['bn1_v'] + eps)
    t1 = inp['bn1_b'] - inp['bn1_m'] * s1
    s2 = inp['bn2_g'] / np.sqrt(inp['bn2_v'] + eps)
    t2 = inp['bn2_b'] - inp['bn2_m'] * s2
    W1 = inp['kq1_w'] * s1[None, :]
    b1 = inp['kq1_b'] + inp['kq1_w'] @ t1
    W2 = inp['kq2_w'] * s2[None, :]
    b2 = inp['kq2_b'] + inp['kq2_w'] @ t2
    sl = inp['bnl_g'] / np.sqrt(inp['bnl_v'] + eps)
    tl = inp['bnl_b'] - inp['bnl_m'] * sl
    ws = inp['w_scale'][0]
    Wout = (ws * sl)[:, None] * inp['out_w']
    bout_f = ws * (sl * inp['out_b'] + tl)

    wkq1 = np.ascontiguousarray(W1.T.reshape(2, 128, 128), dtype=f32)
    wkq2 = np.ascontiguousarray(W2.T.reshape(2, 128, 128), dtype=f32)
    wv = np.ascontiguousarray(inp['v_w'].T.reshape(2, 128, 128), dtype=f32)
    wout_a = np.ascontiguousarray(Wout.T, dtype=f32)
    bq = np.concatenate([b1[32:64], b2[32:64], b1[96:128], b2[96:128]]
                        ).reshape(128, 1).astype(f32)
    bv = inp['v_b'].reshape(128, 1).astype(f32)
    bout_a = bout_f.reshape(2, 128, 1).astype(f32)

    shared = dict(wkq1=wkq1, wkq2=wkq2, wv=wv, wout=wout_a, bq=bq, bv=bv,
                  bout=bout_a, onesd=np.ones((128, 64), dtype=f32))

    in_maps = []
    for b in range(4):
        x1Tb = inp['x1'][b].reshape(C, N)
        x2Tb = inp['x2'][b].reshape(C, N)
        xTb = inp['x'][b].reshape(C, N)
        for qh in range(2):
            if qh == 0:
                m = dict(x1T=np.ascontiguousarray(x1Tb),
                         x2T=np.ascontiguousarray(x2Tb),
                         xT=np.ascontiguousarray(xTb))
            else:
                m = dict(x1T=np.roll(x1Tb, -NQ, axis=1),
                         x2T=np.roll(x2Tb, -NQ, axis=1),
                         xT=np.roll(xTb, -NQ, axis=1))
            m.update(shared)
            in_maps.append(m)

    nc = _get_nc()
    res = run_bass_kernel_spmd(nc, in_maps, list(range(8)), trace=trace)

    out = np.empty((4, C, 64, 64), dtype=f32)
    for b in range(4):
        full = np.empty((C, N), dtype=f32)
        full[:, 0:NQ] = res.results[2 * b]["outT"]
        full[:, NQ:N] = res.results[2 * b + 1]["outT"]
        out[b] = full.reshape(C, 64, 64)
    return out, res



# revision 2
# speedup vs baseline: 1.1916x; 1.1916x over previous
"""Cross_Atten_Lite_split Trainium2 Bass kernel (v2 — pipelined).

Sharding: 8 cores = (batch b in 0..3) x (query-half qh in 0..1).
Each core computes both attention heads for 2048 queries x 4096 keys of
its batch. No collectives. Math rewrites (validated vs reference):
  - eval-mode BN on x1/x2 folded into kq1_w/kq2_w (+bias).
  - channel_shuffle is a permutation of the shared q/k contraction axis
    -> eliminated;  k_h = [kq1[:,64h:64h+32]; kq2[:,64h:64h+32]],
    q_h likewise from rows 64h+32:64h+64.
  - K bias cancels in softmax (adds a per-query-row constant); dropped.
  - final BN + w_scale folded into out_w/out_b.
  - softmax without max-subtraction (max |score| ~ 67.5 < 88, fp32 safe).
  - softmax denominator via ones-augmented V (row 64 of PV output).

v2 structure (single software-pipelined stream):
  - Input DMA issued tile-major; attention groups for the j=0 query tile
    execute inside the DMA/projection window so PE never idles.
  - S matmuls run 2 groups ahead of their PV consumers in PE program
    order, so exp latency (Act engine) is hidden.
  - softmax exp is split across three engines: true Exp on Act, and a
    Schraudolph fast-exp (int32(x*A+B) bitcast to f32, one tensor_scalar)
    on Pool and DVE for a subset of groups.  Weight error of the approx
    exp is ~2-3% on those groups' softmax weights, well inside the 2e-2
    gate (measured end-to-end rel err stays < 2e-3).
  - K scatter + Q bias on Pool, V bias + Vtok scatter on DVE, so the Act
    engine only runs exp.
"""

import numpy as np
from contextlib import ExitStack

import concourse.bass as bass
import concourse.bacc as bacc
import concourse.mybir as mybir
import concourse.tile as tile
from concourse.bass_utils import run_bass_kernel_spmd
from concourse.masks import make_identity

F32 = mybir.dt.float32
F32R = mybir.dt.float32r
I32 = mybir.dt.int32
AF = mybir.ActivationFunctionType
ALU = bass.mybir.AluOpType

C = 256          # channels (INC1 == INC2)
N = 4096         # tokens per batch (64*64)
NQ = 2048        # queries per core
NT = 512         # free-dim tile size

# Schraudolph fast-exp: exp(s*0.125) ~ bitcast(int32(s*FE_A + FE_B))
FE_A = 0.125 * (2 ** 23) / float(np.log(2.0))
FE_B = 127.0 * (2 ** 23) - 486411.0

# exp engine per group: stream of 128 groups; first 32 (j=0, in the DMA
# window, Pool busy with scatters) -> Act; rest cycle this pattern.
_POST_PATTERN = ['act', 'pool', 'act', 'pool', 'act', 'dve', 'act', 'pool']


def _exp_engine(i):
    if i < 32:
        return 'act'
    return _POST_PATTERN[(i - 32) % len(_POST_PATTERN)]


def build_bass():
    nc = bacc.Bacc("TRN2", target_bir_lowering=False, debug=False, num_devices=8)

    x1T = nc.dram_tensor("x1T", [C, N], F32R, kind="ExternalInput").ap()
    x2T = nc.dram_tensor("x2T", [C, N], F32R, kind="ExternalInput").ap()
    xT = nc.dram_tensor("xT", [C, N], F32R, kind="ExternalInput").ap()
    wkq1 = nc.dram_tensor("wkq1", [2, 128, 128], F32R, kind="ExternalInput").ap()
    wkq2 = nc.dram_tensor("wkq2", [2, 128, 128], F32R, kind="ExternalInput").ap()
    wv = nc.dram_tensor("wv", [2, 128, 128], F32R, kind="ExternalInput").ap()
    wout = nc.dram_tensor("wout", [128, 256], F32R, kind="ExternalInput").ap()
    bq = nc.dram_tensor("bq", [128, 1], F32, kind="ExternalInput").ap()
    bv = nc.dram_tensor("bv", [128, 1], F32, kind="ExternalInput").ap()
    bout = nc.dram_tensor("bout", [2, 128, 1], F32, kind="ExternalInput").ap()
    outT = nc.dram_tensor("outT", [C, NQ], F32, kind="ExternalOutput").ap()

    with ExitStack() as ctx:
        tc = ctx.enter_context(tile.TileContext(nc))
        const = ctx.enter_context(tc.tile_pool(name="const", bufs=1))
        pers = ctx.enter_context(tc.tile_pool(name="pers", bufs=1))

        # constants
        w_kq1 = [const.tile([128, 128], F32R, name=f"wkq1_{g}") for g in range(2)]
        w_kq2 = [const.tile([128, 128], F32R, name=f"wkq2_{g}") for g in range(2)]
        w_v = [const.tile([128, 128], F32R, name=f"wv_{g}") for g in range(2)]
        w_out = const.tile([128, 256], F32R, name="wout")
        b_q = const.tile([128, 1], F32, name="bq")
        b_v = const.tile([128, 1], F32, name="bv")
        b_out = [const.tile([128, 1], F32, name=f"bout_{g}") for g in range(2)]
        ident = const.tile([128, 128], F32, name="ident")
        ones1 = const.tile([1, 64], F32R, name="ones1")

        for g in range(2):
            nc.sync.dma_start(out=w_kq1[g][:], in_=wkq1[g])
            nc.sync.dma_start(out=w_kq2[g][:], in_=wkq2[g])
            nc.sync.dma_start(out=w_v[g][:], in_=wv[g])
            nc.sync.dma_start(out=b_out[g][:], in_=bout[g])
        nc.sync.dma_start(out=w_out[:], in_=wout[:])
        nc.sync.dma_start(out=b_q[:], in_=bq[:])
        nc.sync.dma_start(out=b_v[:], in_=bv[:])
        make_identity(nc, ident[:])
        nc.vector.memset(ones1[:], 1.0)

        # persistent SBUF
        KT = pers.tile([128, N], F32R, name="KT")      # rows k1a,k1b,k2a,k2b
        QT = pers.tile([128, NQ], F32R, name="QT")     # rows q1a,q1b,q2a,q2b
        Vtok = pers.tile([128, 32 * 130], F32R, name="Vtok")
        xsb = [pers.tile([128, N], F32R, name=f"xsb_{g}") for g in range(2)]
        x1sb = [pers.tile([128, N], F32R, name=f"x1sb_{g}") for g in range(2)]
        x2sb = [pers.tile([128, N], F32R, name=f"x2sb_{g}") for g in range(2)]
        Ocat = pers.tile([128, NQ], F32R, name="Ocat")

        # ones-columns of each Vtok m-block (softmax denominator rows)
        vtok3 = Vtok.rearrange("p (m c) -> p m c", c=130)
        nc.vector.memset(vtok3[:, :, 64:65], 1.0)
        nc.vector.memset(vtok3[:, :, 129:130], 1.0)

        # SBUF pools
        pvt = ctx.enter_context(tc.tile_pool(name="pvt", bufs=2))
        poolE = ctx.enter_context(tc.tile_pool(name="poolE", bufs=3))
        small = ctx.enter_context(tc.tile_pool(name="small", bufs=2))
        pout = ctx.enter_context(tc.tile_pool(name="pout", bufs=4))
        # PSUM pools live across both phases
        poolS = ctx.enter_context(tc.tile_pool(name="poolS", bufs=2, space="PSUM"))
        poolO = ctx.enter_context(tc.tile_pool(name="poolO", bufs=2, space="PSUM"))

        # ---------------- stream definition ----------------
        # group i -> (h, j, g): j-major, g inner, heads interleaved.
        STREAM = [(h, j, g) for j in range(4) for g in range(16) for h in range(2)]
        NG = len(STREAM)  # 128
        LOOK = 2          # S/exp runs LOOK groups ahead of PV

        state = {
            'emitted': 0,       # S/exp emitted up to this stream index
            'pv': 0,            # PV emitted up to this stream index
            'sp': {},           # stream idx -> sp psum tile
            'et': {},           # stream idx -> et sbuf tile
            'op': {},           # (h, j) -> op psum tile
            'poolR': None,
            'poolC': None,
        }

        def emit_S_exp(i):
            h, j, g = STREAM[i]
            hs = slice(64 * h, 64 * (h + 1))
            qs = slice(j * NT, (j + 1) * NT)
            sp = poolS.tile([128, 2 * NT], F32, tag="sp", name=f"sp_{h}_{j}_{g}")
            for b2 in range(2):
                m = 2 * g + b2
                nc.tensor.matmul(
                    sp[:, b2 * NT:(b2 + 1) * NT],
                    KT[hs, m * 128:(m + 1) * 128],
                    QT[hs, qs],
                    start=True, stop=True)
            et = poolE.tile([128, 2 * NT], F32R, tag="et", name=f"et_{h}_{j}_{g}")
            eng = _exp_engine(i)
            if eng == 'act':
                nc.scalar.activation(et[:], sp[:], AF.Exp, scale=0.125)
            else:
                e = nc.gpsimd if eng == 'pool' else nc.vector
                with nc.allow_low_precision(reason="schraudolph exp"):
                    e.tensor_scalar(et[:].bitcast(I32), sp[:], FE_A, FE_B,
                                    op0=ALU.mult, op1=ALU.add)
            state['sp'][i] = sp
            state['et'][i] = et

        def emit_PV(i):
            h, j, g = STREAM[i]
            qs = slice(j * NT, (j + 1) * NT)
            if g == 0:
                state['op'][(h, j)] = poolO.tile(
                    [65, NT], F32, tag="op", name=f"op_{h}_{j}")
            op = state['op'][(h, j)]
            et = state['et'].pop(i)
            state['sp'].pop(i)
            for b2 in range(2):
                m = 2 * g + b2
                nc.tensor.matmul(
                    op[:],
                    Vtok[:, m * 130 + 65 * h:m * 130 + 65 * h + 65],
                    et[:, b2 * NT:(b2 + 1) * NT],
                    start=(m == 0),
                    stop=(m == 31))
            if g == 15:
                emit_drain(h, j)
                if h == 1:
                    emit_C(j)

        def emit_drain(h, j):
            hs = slice(64 * h, 64 * (h + 1))
            qs = slice(j * NT, (j + 1) * NT)
            op = state['op'][(h, j)]
            rec = small.tile([1, NT], F32R, tag="rec", name=f"rec_{h}_{j}")
            with nc.allow_low_precision(reason="f32r is fp32 bits"):
                nc.vector.reciprocal(rec[:], op[64:65, :])
            rb = state['poolR'].tile([64, NT], F32, tag="rb", name=f"rb_{h}_{j}")
            nc.tensor.matmul(rb[:], ones1[:], rec[:], start=True, stop=True)
            rbs = small.tile([64, NT], F32, tag="rbs", name=f"rbs_{h}_{j}")
            nc.gpsimd.tensor_copy(rbs[:], rb[:])
            nc.vector.tensor_mul(Ocat[hs, qs], op[0:64, :], rbs[:])
            del state['op'][(h, j)]

        def emit_C(j):
            qs = slice(j * NT, (j + 1) * NT)
            for g in range(2):
                pp = state['poolC'].tile([128, NT], F32, tag="pp", name=f"pp_{j}_{g}")
                nc.tensor.matmul(pp[:], w_out[:, 128 * g:128 * (g + 1)],
                                 Ocat[:, qs], start=True, stop=True)
                osb = pout.tile([128, NT], F32, tag="osb", name=f"osb_{j}_{g}")
                nc.vector.scalar_tensor_tensor(
                    osb[:], pp[:], b_out[g][:], xsb[g][:, qs].bitcast(F32),
                    op0=ALU.add, op1=ALU.add)
                nc.sync.dma_start(out=outT[128 * g:128 * (g + 1), qs], in_=osb[:])

        def pump(limit):
            """Emit S/exp up to stream index `limit`; PV trails by LOOK."""
            while state['emitted'] < limit:
                i = state['emitted']
                emit_S_exp(i)
                state['emitted'] = i + 1
                if i - LOOK >= 0:
                    emit_PV(i - LOOK)
                    state['pv'] = i - LOOK + 1

        def drain_stream():
            while state['pv'] < NG:
                emit_PV(state['pv'])
                state['pv'] += 1

        # ---------------- phase A window (+ j=0 attention) ----------------
        with ExitStack() as actx:
            poolA = actx.enter_context(tc.tile_pool(name="poolA", bufs=2, space="PSUM"))

            for t in range(8):
                cs = slice(t * NT, (t + 1) * NT)
                # input DMA for this tile (kq inputs first)
                for g in range(2):
                    nc.sync.dma_start(out=x1sb[g][:, cs], in_=x1T[128 * g:128 * (g + 1), cs])
                for g in range(2):
                    nc.sync.dma_start(out=x2sb[g][:, cs], in_=x2T[128 * g:128 * (g + 1), cs])
                for g in range(2):
                    nc.sync.dma_start(out=xsb[g][:, cs], in_=xT[128 * g:128 * (g + 1), cs])

                # B-groups unlocked by tile t-1: stream prefix 4(t-1)+4 = 4t
                pump(min(4 * t, 32))

                # projections for tile t
                kq1p = poolA.tile([128, NT], F32, tag="mmA", name=f"kq1p_{t}")
                nc.tensor.matmul(kq1p[:], w_kq1[0][:], x1sb[0][:, cs], start=True, stop=False)
                nc.tensor.matmul(kq1p[:], w_kq1[1][:], x1sb[1][:, cs], start=False, stop=True)
                kq2p = poolA.tile([128, NT], F32, tag="mmA", name=f"kq2p_{t}")
                nc.tensor.matmul(kq2p[:], w_kq2[0][:], x2sb[0][:, cs], start=True, stop=False)
                nc.tensor.matmul(kq2p[:], w_kq2[1][:], x2sb[1][:, cs], start=False, stop=True)

                # K scatter (+Q bias) on Pool, straight from PSUM
                nc.gpsimd.tensor_copy(KT[0:32, cs], kq1p[0:32, :])
                nc.gpsimd.tensor_copy(KT[32:64, cs], kq2p[0:32, :])
                nc.gpsimd.tensor_copy(KT[64:96, cs], kq1p[64:96, :])
                nc.gpsimd.tensor_copy(KT[96:128, cs], kq2p[64:96, :])
                if t < 4:  # query half
                    nc.gpsimd.tensor_scalar(QT[0:32, cs], kq1p[32:64, :],
                                            b_q[0:32, :], None, op0=ALU.add)
                    nc.gpsimd.tensor_scalar(QT[32:64, cs], kq2p[32:64, :],
                                            b_q[32:64, :], None, op0=ALU.add)
                    nc.gpsimd.tensor_scalar(QT[64:96, cs], kq1p[96:128, :],
                                            b_q[64:96, :], None, op0=ALU.add)
                    nc.gpsimd.tensor_scalar(QT[96:128, cs], kq2p[96:128, :],
                                            b_q[96:128, :], None, op0=ALU.add)

                # keep PE busy while Pool drains kq1p/kq2p readers
                pump(min(4 * t + 2, 32))

                vp = poolA.tile([128, NT], F32, tag="mmA", name=f"vp_{t}")
                nc.tensor.matmul(vp[:], w_v[0][:], xsb[0][:, cs], start=True, stop=False)
                nc.tensor.matmul(vp[:], w_v[1][:], xsb[1][:, cs], start=False, stop=True)
                VT = pvt.tile([128, NT], F32, tag="VT", name=f"VT_{t}")
                nc.vector.tensor_scalar(VT[:], vp[:], b_v[:], None, op0=ALU.add)

                # transpose V for PV matmuls: one psum tile, 4 blocks
                tpw = poolA.tile([128, NT], F32, tag="mmA", name=f"tpw_{t}")
                for s in range(4):
                    nc.tensor.transpose(tpw[:, s * 128:(s + 1) * 128],
                                        VT[:, s * 128:(s + 1) * 128], ident[:])
                # scatter into Vtok (cols 0:64 = v1, 65:129 = v2) in 2 copies
                tp4 = tpw.rearrange("p (s c) -> p s c", c=128)
                vt4 = vtok3[:, 4 * t:4 * t + 4, :]
                nc.vector.tensor_copy(vt4[:, :, 0:64], tp4[:, :, 0:64])
                nc.vector.tensor_copy(vt4[:, :, 65:129], tp4[:, :, 64:128])

        # ---------------- post-window: rest of attention + output ----------
        with ExitStack() as bctx:
            state['poolR'] = bctx.enter_context(
                tc.tile_pool(name="poolR", bufs=1, space="PSUM"))
            state['poolC'] = bctx.enter_context(
                tc.tile_pool(name="poolC", bufs=1, space="PSUM"))
            pump(NG)
            drain_stream()

    nc.compile()
    return nc


_NC = None


def _get_nc():
    global _NC
    if _NC is None:
        _NC = build_bass()
    return _NC


def kernel(**inputs):
    out, _ = _run(inputs, trace=False)
    return out


def _run(inputs, trace=False):
    eps = 1e-5
    f32 = np.float32
    inp = {k: np.asarray(v, dtype=np.float32) for k, v in inputs.items()}

    s1 = inp['bn1_g'] / np.sqrt(inp['bn1_v'] + eps)
    t1 = inp['bn1_b'] - inp['bn1_m'] * s1
    s2 = inp['bn2_g'] / np.sqrt(inp['bn2_v'] + eps)
    t2 = inp['bn2_b'] - inp['bn2_m'] * s2
    W1 = inp['kq1_w'] * s1[None, :]
    b1 = inp['kq1_b'] + inp['kq1_w'] @ t1
    W2 = inp['kq2_w'] * s2[None, :]
    b2 = inp['kq2_b'] + inp['kq2_w'] @ t2
    sl = inp['bnl_g'] / np.sqrt(inp['bnl_v'] + eps)
    tl = inp['bnl_b'] - inp['bnl_m'] * sl
    ws = inp['w_scale'][0]
    Wout = (ws * sl)[:, None] * inp['out_w']
    bout_f = ws * (sl * inp['out_b'] + tl)

    wkq1 = np.ascontiguousarray(W1.T.reshape(2, 128, 128), dtype=f32)
    wkq2 = np.ascontiguousarray(W2.T.reshape(2, 128, 128), dtype=f32)
    wv = np.ascontiguousarray(inp['v_w'].T.reshape(2, 128, 128), dtype=f32)
    wout_a = np.ascontiguousarray(Wout.T, dtype=f32)
    bq = np.concatenate([b1[32:64], b2[32:64], b1[96:128], b2[96:128]]
                        ).reshape(128, 1).astype(f32)
    bv = inp['v_b'].reshape(128, 1).astype(f32)
    bout_a = bout_f.reshape(2, 128, 1).astype(f32)

    shared = dict(wkq1=wkq1, wkq2=wkq2, wv=wv, wout=wout_a, bq=bq, bv=bv,
                  bout=bout_a)

    in_maps = []
    for b in range(4):
        x1Tb = inp['x1'][b].reshape(C, N)
        x2Tb = inp['x2'][b].reshape(C, N)
        xTb = inp['x'][b].reshape(C, N)
        for qh in range(2):
            if qh == 0:
                m = dict(x1T=np.ascontiguousarray(x1Tb),
                         x2T=np.ascontiguousarray(x2Tb),
                         xT=np.ascontiguousarray(xTb))
            else:
                m = dict(x1T=np.roll(x1Tb, -NQ, axis=1),
                         x2T=np.roll(x2Tb, -NQ, axis=1),
                         xT=np.roll(xTb, -NQ, axis=1))
            m.update(shared)
            in_maps.append(m)

    nc = _get_nc()
    res = run_bass_kernel_spmd(nc, in_maps, list(range(8)), trace=trace)

    out = np.empty((4, C, 64, 64), dtype=f32)
    for b in range(4):
        full = np.empty((C, N), dtype=f32)
        full[:, 0:NQ] = res.results[2 * b]["outT"]
        full[:, NQ:N] = res.results[2 * b + 1]["outT"]
        out[b] = full.reshape(C, 64, 64)
    return out, res


# revision 6
# speedup vs baseline: 1.3444x; 1.1283x over previous
"""Cross_Atten_Lite_split Trainium2 Bass kernel (v3 — pipelined).

Sharding: 8 cores = (batch b in 0..3) x (query-half qh in 0..1).
Each core computes both attention heads for 2048 queries x 4096 keys of
its batch. No collectives. Math rewrites (validated vs reference):
  - eval-mode BN on x1/x2 folded into kq1_w/kq2_w (+bias).
  - channel_shuffle is a permutation of the shared q/k contraction axis
    -> eliminated;  k_h = [kq1[:,64h:64h+32]; kq2[:,64h:64h+32]],
    q_h likewise from rows 64h+32:64h+64.
  - K bias cancels in softmax (adds a per-query-row constant); dropped.
  - final BN + w_scale folded into out_w/out_b.
  - softmax without max-subtraction (max |score| ~ 67.5 < 88, fp32 safe).
  - softmax denominator via ones-augmented V (row 64 of PV output).

v3 structure (single software-pipelined stream):
  - All weights land in 2 merged DMAs; inputs in 3 merged DMAs per
    512-column tile (descriptor-gen on HWDGE costs ~650ns per DMA, so
    DMA count matters as much as bytes).
  - Attention groups for the j=0 query tile execute inside the
    DMA/projection window so PE never idles; S matmuls run LOOK=3
    groups ahead of their PV consumers so exp latency is hidden.
  - softmax exp split across three engines: true Exp on Act, and a
    Schraudolph fast-exp (int32(x*A+B) bitcast to f32, one
    tensor_scalar) on Pool and DVE for a subset of groups.  End-to-end
    rel err stays < 2e-3, inside the 2e-2 gate.
  - K scatter on Pool, Q/V bias + Vtok scatter on DVE; drain chains
    (reciprocal/broadcast/normalize) and the output projection are
    emitted a few stream steps late so PE never waits on them.
"""

import numpy as np
from contextlib import ExitStack

import concourse.bass as bass
import concourse.bacc as bacc
import concourse.mybir as mybir
import concourse.tile as tile
from concourse.bass_utils import run_bass_kernel_spmd
from concourse.masks import make_identity

F32 = mybir.dt.float32
F32R = mybir.dt.float32r
I32 = mybir.dt.int32
AF = mybir.ActivationFunctionType
ALU = bass.mybir.AluOpType

C = 256          # channels (INC1 == INC2)
N = 4096         # tokens per batch (64*64)
NQ = 2048        # queries per core
NT = 512         # free-dim tile size

# Schraudolph fast-exp: exp(s*0.125) ~ bitcast(int32(s*FE_A + FE_B))
FE_A = 0.125 * (2 ** 23) / float(np.log(2.0))
FE_B = 127.0 * (2 ** 23) - 486411.0

# exp engine per group: stream of 128 groups; first 32 (j=0, inside the
# DMA window where Pool runs the K scatters) -> Act; rest cycle this.
_POST_PATTERN = ['act', 'pool', 'act', 'dve', 'act', 'pool', 'act', 'pool']


def _exp_engine(i):
    if i < 32:
        return 'act'
    return _POST_PATTERN[(i - 32) % len(_POST_PATTERN)]


def build_bass():
    nc = bacc.Bacc("TRN2", target_bir_lowering=False, debug=False, num_devices=8)

    x1T = nc.dram_tensor("x1T", [C, N], F32R, kind="ExternalInput").ap()
    x2T = nc.dram_tensor("x2T", [C, N], F32R, kind="ExternalInput").ap()
    xT = nc.dram_tensor("xT", [C, N], F32R, kind="ExternalInput").ap()
    wcat_d = nc.dram_tensor("wcat", [128, 1024], F32R, kind="ExternalInput").ap()
    bcat_d = nc.dram_tensor("bcat", [128, 4], F32, kind="ExternalInput").ap()
    outT = nc.dram_tensor("outT", [C, NQ], F32, kind="ExternalOutput").ap()

    x1g = x1T.rearrange("(g p) c -> p g c", p=128)
    x2g = x2T.rearrange("(g p) c -> p g c", p=128)
    xg = xT.rearrange("(g p) c -> p g c", p=128)
    outg = outT.rearrange("(g p) c -> p g c", p=128)

    with ExitStack() as ctx:
        tc = ctx.enter_context(tile.TileContext(nc))
        const = ctx.enter_context(tc.tile_pool(name="const", bufs=1))
        pers = ctx.enter_context(tc.tile_pool(name="pers", bufs=1))

        # merged constants: wcat = [kq1 g0|g1, kq2 g0|g1, v g0|g1, wout]
        wcat = const.tile([128, 1024], F32R, name="wcat")
        bcat = const.tile([128, 4], F32, name="bcat")
        ident = const.tile([128, 128], F32R, name="ident")
        ones1 = const.tile([1, 64], F32R, name="ones1")

        nc.sync.dma_start(out=wcat[:], in_=wcat_d[:])
        nc.sync.dma_start(out=bcat[:], in_=bcat_d[:])
        make_identity(nc, ident[:])
        nc.vector.memset(ones1[:], 1.0)

        w_kq1 = [wcat[:, 128 * g:128 * (g + 1)] for g in range(2)]
        w_kq2 = [wcat[:, 256 + 128 * g:256 + 128 * (g + 1)] for g in range(2)]
        w_v = [wcat[:, 512 + 128 * g:512 + 128 * (g + 1)] for g in range(2)]
        w_out = wcat[:, 768:1024]
        b_q = bcat[:, 0:1]
        b_v = bcat[:, 1:2]
        b_out = [bcat[:, 2 + g:3 + g] for g in range(2)]

        # persistent SBUF
        KT = pers.tile([128, N], F32R, name="KT")      # rows k1a,k2a,k1b,k2b
        QT = pers.tile([128, NQ], F32R, name="QT")     # rows q1a,q2a,q1b,q2b
        Vtok = pers.tile([128, 32 * 130], F32R, name="Vtok")
        xsb = pers.tile([128, 2 * N], F32R, name="xsb")
        x1sb = pers.tile([128, 2 * N], F32R, name="x1sb")
        x2sb = pers.tile([128, 2 * N], F32R, name="x2sb")
        Ocat = pers.tile([128, NQ], F32R, name="Ocat")

        def gsl(g, cs):
            return slice(g * N + cs.start, g * N + cs.stop)

        # ones-columns of each Vtok m-block (softmax denominator rows)
        vtok3 = Vtok.rearrange("p (m c) -> p m c", c=130)
        nc.vector.memset(vtok3[:, :, 64:65], 1.0)
        nc.vector.memset(vtok3[:, :, 129:130], 1.0)

        # SBUF pools
        pvt = ctx.enter_context(tc.tile_pool(name="pvt", bufs=2))
        poolE = ctx.enter_context(tc.tile_pool(name="poolE", bufs=5))
        small = ctx.enter_context(tc.tile_pool(name="small", bufs=2))
        pout = ctx.enter_context(tc.tile_pool(name="pout", bufs=2))
        # PSUM pools live across both phases
        poolS = ctx.enter_context(tc.tile_pool(name="poolS", bufs=2, space="PSUM"))
        poolO = ctx.enter_context(tc.tile_pool(name="poolO", bufs=2, space="PSUM"))

        # ---------------- stream definition ----------------
        # group i -> (h, j, g): j-major, g inner, heads interleaved.
        STREAM = [(h, j, g) for j in range(4) for g in range(16) for h in range(2)]
        NG = len(STREAM)  # 128
        LOOK = 3          # S/exp runs LOOK groups ahead of PV

        state = {
            'emitted': 0,       # S/exp emitted up to this stream index
            'pv': 0,            # PV emitted up to this stream index
            'step': 0,          # emission step counter (for deferred work)
            'pending': [],      # (due_step, fn) deferred emissions
            'sp': {},           # stream idx -> sp psum tile
            'et': {},           # stream idx -> et sbuf tile
            'op': {},           # (h, j) -> op psum tile
            'poolR': None,
            'poolC': None,
        }

        def flush(now):
            keep = []
            for due, fn in state['pending']:
                if due <= now:
                    fn()
                else:
                    keep.append((due, fn))
            state['pending'] = keep

        def defer(delta, fn):
            state['pending'].append((state['step'] + delta, fn))

        def emit_S_exp(i):
            h, j, g = STREAM[i]
            hs = slice(64 * h, 64 * (h + 1))
            qs = slice(j * NT, (j + 1) * NT)
            sp = poolS.tile([128, 2 * NT], F32, tag="sp", name=f"sp_{h}_{j}_{g}")
            for b2 in range(2):
                m = 2 * g + b2
                nc.tensor.matmul(
                    sp[:, b2 * NT:(b2 + 1) * NT],
                    KT[hs, m * 128:(m + 1) * 128],
                    QT[hs, qs],
                    start=True, stop=True)
            et = poolE.tile([128, 2 * NT], F32R, tag="et", name=f"et_{h}_{j}_{g}")
            eng = _exp_engine(i)
            if eng == 'act':
                nc.scalar.activation(et[:], sp[:], AF.Exp, scale=0.125)
            else:
                e = nc.gpsimd if eng == 'pool' else nc.vector
                with nc.allow_low_precision(reason="schraudolph exp"):
                    e.tensor_scalar(et[:].bitcast(I32), sp[:], FE_A, FE_B,
                                    op0=ALU.mult, op1=ALU.add)
            state['sp'][i] = sp
            state['et'][i] = et

        def emit_PV(i):
            h, j, g = STREAM[i]
            if g == 0:
                state['op'][(h, j)] = poolO.tile(
                    [65, NT], F32, tag="op", name=f"op_{h}_{j}")
            op = state['op'][(h, j)]
            et = state['et'].pop(i)
            state['sp'].pop(i)
            for b2 in range(2):
                m = 2 * g + b2
                nc.tensor.matmul(
                    op[:],
                    Vtok[:, m * 130 + 65 * h:m * 130 + 65 * h + 65],
                    et[:, b2 * NT:(b2 + 1) * NT],
                    start=(m == 0),
                    stop=(m == 31))
            if g == 15:
                # reciprocal can start immediately (DVE); the PE/Pool/DVE
                # pieces of the drain are deferred so PE's queue never
                # blocks on them.
                hh, jj = h, j
                hsl = slice(64 * h, 64 * (h + 1))
                qsl = slice(j * NT, (j + 1) * NT)
                op_t = op
                rec = small.tile([1, NT], F32R, tag="rec", name=f"rec_{h}_{j}")
                with nc.allow_low_precision(reason="f32r is fp32 bits"):
                    nc.vector.reciprocal(rec[:], op_t[64:65, :])

                def drain():
                    rb = state['poolR'].tile([64, NT], F32, tag="rb",
                                             name=f"rb_{hh}_{jj}")
                    nc.tensor.matmul(rb[:], ones1[:], rec[:], start=True, stop=True)
                    rbs = small.tile([64, NT], F32, tag="rbs", name=f"rbs_{hh}_{jj}")
                    nc.gpsimd.tensor_copy(rbs[:], rb[:])
                    nc.vector.tensor_mul(Ocat[hsl, qsl], op_t[0:64, :], rbs[:])
                    del state['op'][(hh, jj)]

                defer(2, drain)
                if h == 1:
                    defer(3, lambda: emit_C0(jj))
                    defer(4, lambda: emit_C1(jj))

        def emit_C0(j):
            qs = slice(j * NT, (j + 1) * NT)
            pp = state['poolC'].tile([128, NT], F32, tag="pp", name=f"pp_{j}_0")
            nc.tensor.matmul(pp[:], w_out[:, 0:128], Ocat[:, qs],
                             start=True, stop=True)
            state[f'pp0_{j}'] = pp

        def emit_C1(j):
            qs = slice(j * NT, (j + 1) * NT)
            osb = pout.tile([128, 2 * NT], F32, tag="osb", name=f"osb_{j}")
            pp0 = state.pop(f'pp0_{j}')
            nc.vector.scalar_tensor_tensor(
                osb[:, 0:NT], pp0[:], b_out[0][:], xsb[:, gsl(0, qs)].bitcast(F32),
                op0=ALU.add, op1=ALU.add)
            pp = state['poolC'].tile([128, NT], F32, tag="pp", name=f"pp_{j}_1")
            nc.tensor.matmul(pp[:], w_out[:, 128:256], Ocat[:, qs],
                             start=True, stop=True)
            nc.vector.scalar_tensor_tensor(
                osb[:, NT:2 * NT], pp[:], b_out[1][:], xsb[:, gsl(1, qs)].bitcast(F32),
                op0=ALU.add, op1=ALU.add)
            nc.sync.dma_start(out=outg[:, :, qs],
                              in_=osb.rearrange("p (g c) -> p g c", c=NT))

        def pump(limit):
            """Emit S/exp up to stream index `limit`; PV trails by LOOK."""
            while state['emitted'] < limit:
                i = state['emitted']
                flush(state['step'])
                emit_S_exp(i)
                state['emitted'] = i + 1
                state['step'] += 1
                if i - LOOK >= 0:
                    emit_PV(i - LOOK)
                    state['pv'] = i - LOOK + 1

        def drain_stream():
            while state['pv'] < NG:
                flush(state['step'])
                emit_PV(state['pv'])
                state['pv'] += 1
                state['step'] += 1
            state['step'] += 100
            flush(state['step'])

        # ---------------- phase A window (+ j=0 attention) ----------------
        with ExitStack() as actx:
            poolA = actx.enter_context(tc.tile_pool(name="poolA", bufs=2, space="PSUM"))

            for t in range(8):
                cs = slice(t * NT, (t + 1) * NT)
                # merged input DMA for this tile (kq inputs first)
                nc.sync.dma_start(
                    out=x1sb.rearrange("p (g c) -> p g c", c=N)[:, :, cs],
                    in_=x1g[:, :, cs])
                nc.sync.dma_start(
                    out=x2sb.rearrange("p (g c) -> p g c", c=N)[:, :, cs],
                    in_=x2g[:, :, cs])
                nc.sync.dma_start(
                    out=xsb.rearrange("p (g c) -> p g c", c=N)[:, :, cs],
                    in_=xg[:, :, cs])

                # projections for tile t
                kq1p = poolA.tile([128, NT], F32, tag="mmA", name=f"kq1p_{t}")
                nc.tensor.matmul(kq1p[:], w_kq1[0], x1sb[:, gsl(0, cs)], start=True, stop=False)
                nc.tensor.matmul(kq1p[:], w_kq1[1], x1sb[:, gsl(1, cs)], start=False, stop=True)
                kq2p = poolA.tile([128, NT], F32, tag="mmA", name=f"kq2p_{t}")
                nc.tensor.matmul(kq2p[:], w_kq2[0], x2sb[:, gsl(0, cs)], start=True, stop=False)
                nc.tensor.matmul(kq2p[:], w_kq2[1], x2sb[:, gsl(1, cs)], start=False, stop=True)

                # K scatter on Pool (kq1p consumers first so its psum slot
                # frees early), Q bias on DVE
                nc.gpsimd.tensor_copy(KT[0:32, cs], kq1p[0:32, :])
                nc.gpsimd.tensor_copy(KT[64:96, cs], kq1p[64:96, :])
                nc.gpsimd.tensor_copy(KT[32:64, cs], kq2p[0:32, :])
                nc.gpsimd.tensor_copy(KT[96:128, cs], kq2p[64:96, :])
                if t < 4:  # query half
                    nc.vector.tensor_scalar(QT[0:32, cs], kq1p[32:64, :],
                                            b_q[0:32, :], None, op0=ALU.add)
                    nc.vector.tensor_scalar(QT[64:96, cs], kq1p[96:128, :],
                                            b_q[64:96, :], None, op0=ALU.add)
                    nc.vector.tensor_scalar(QT[32:64, cs], kq2p[32:64, :],
                                            b_q[32:64, :], None, op0=ALU.add)
                    nc.vector.tensor_scalar(QT[96:128, cs], kq2p[96:128, :],
                                            b_q[96:128, :], None, op0=ALU.add)

                # older groups keep PE busy while Pool/DVE drain the kq PSUMs
                pump(4 * t)

                vp = poolA.tile([128, NT], F32, tag="mmA", name=f"vp_{t}")
                nc.tensor.matmul(vp[:], w_v[0], xsb[:, gsl(0, cs)], start=True, stop=False)
                nc.tensor.matmul(vp[:], w_v[1], xsb[:, gsl(1, cs)], start=False, stop=True)
                VT = pvt.tile([128, NT], F32R, tag="VT", name=f"VT_{t}")
                nc.vector.tensor_scalar(VT[:], vp[:], b_v[:], None, op0=ALU.add)

                # newest groups (need this tile's K) before the transposes
                pump(4 * t + 2)

                # transpose V for PV matmuls: one psum tile, 4 blocks
                tpw = poolA.tile([128, NT], F32R, tag="mmA", name=f"tpw_{t}")
                for s in range(4):
                    nc.tensor.transpose(tpw[:, s * 128:(s + 1) * 128],
                                        VT[:, s * 128:(s + 1) * 128], ident[:])
                # scatter into Vtok (cols 0:64 = v1, 65:129 = v2) in 2 copies
                tp4 = tpw.rearrange("p (s c) -> p s c", c=128)
                vt4 = vtok3[:, 4 * t:4 * t + 4, :]
                nc.vector.tensor_copy(vt4[:, :, 0:64], tp4[:, :, 0:64])
                nc.vector.tensor_copy(vt4[:, :, 65:129], tp4[:, :, 64:128])

        # ---------------- post-window: rest of attention + output ----------
        with ExitStack() as bctx:
            state['poolR'] = bctx.enter_context(
                tc.tile_pool(name="poolR", bufs=1, space="PSUM"))
            state['poolC'] = bctx.enter_context(
                tc.tile_pool(name="poolC", bufs=1, space="PSUM"))
            pump(NG)
            drain_stream()

    nc.compile()
    return nc


_NC = None


def _get_nc():
    global _NC
    if _NC is None:
        _NC = build_bass()
    return _NC


def kernel(**inputs):
    out, _ = _run(inputs, trace=False)
    return out


def _run(inputs, trace=False):
    eps = 1e-5
    f32 = np.float32
    inp = {k: np.asarray(v, dtype=np.float32) for k, v in inputs.items()}

    s1 = inp['bn1_g'] / np.sqrt(inp['bn1_v'] + eps)
    t1 = inp['bn1_b'] - inp['bn1_m'] * s1
    s2 = inp['bn2_g'] / np.sqrt(inp['bn2_v'] + eps)
    t2 = inp['bn2_b'] - inp['bn2_m'] * s2
    W1 = inp['kq1_w'] * s1[None, :]
    b1 = inp['kq1_b'] + inp['kq1_w'] @ t1
    W2 = inp['kq2_w'] * s2[None, :]
    b2 = inp['kq2_b'] + inp['kq2_w'] @ t2
    sl = inp['bnl_g'] / np.sqrt(inp['bnl_v'] + eps)
    tl = inp['bnl_b'] - inp['bnl_m'] * sl
    ws = inp['w_scale'][0]
    Wout = (ws * sl)[:, None] * inp['out_w']
    bout_f = ws * (sl * inp['out_b'] + tl)

    # merged weights: [kq1 g0|g1, kq2 g0|g1, v g0|g1, wout] as stationaries
    wcat = np.concatenate([
        W1.T[0:128], W1.T[128:256],
        W2.T[0:128], W2.T[128:256],
        inp['v_w'].T[0:128], inp['v_w'].T[128:256],
        Wout.T,
    ], axis=1).astype(f32)
    bq = np.concatenate([b1[32:64], b2[32:64], b1[96:128], b2[96:128]])
    bcat = np.stack([bq, inp['v_b'], bout_f[0:128], bout_f[128:256]],
                    axis=1).astype(f32)

    shared = dict(wcat=np.ascontiguousarray(wcat),
                  bcat=np.ascontiguousarray(bcat))

    in_maps = []
    for b in range(4):
        x1Tb = inp['x1'][b].reshape(C, N)
        x2Tb = inp['x2'][b].reshape(C, N)
        xTb = inp['x'][b].reshape(C, N)
        for qh in range(2):
            if qh == 0:
                m = dict(x1T=np.ascontiguousarray(x1Tb),
                         x2T=np.ascontiguousarray(x2Tb),
                         xT=np.ascontiguousarray(xTb))
            else:
                m = dict(x1T=np.roll(x1Tb, -NQ, axis=1),
                         x2T=np.roll(x2Tb, -NQ, axis=1),
                         xT=np.roll(xTb, -NQ, axis=1))
            m.update(shared)
            in_maps.append(m)

    nc = _get_nc()
    res = run_bass_kernel_spmd(nc, in_maps, list(range(8)), trace=trace)

    out = np.empty((4, C, 64, 64), dtype=f32)
    for b in range(4):
        full = np.empty((C, N), dtype=f32)
        full[:, 0:NQ] = res.results[2 * b]["outT"]
        full[:, NQ:N] = res.results[2 * b + 1]["outT"]
        out[b] = full.reshape(C, 64, 64)
    return out, res


# revision 17
# speedup vs baseline: 1.4195x; 1.0558x over previous
"""Cross_Atten_Lite_split Trainium2 Bass kernel (v3 — pipelined).

Sharding: 8 cores = (batch b in 0..3) x (query-half qh in 0..1).
Each core computes both attention heads for 2048 queries x 4096 keys of
its batch. No collectives. Math rewrites (validated vs reference):
  - eval-mode BN on x1/x2 folded into kq1_w/kq2_w (+bias).
  - channel_shuffle is a permutation of the shared q/k contraction axis
    -> eliminated;  k_h = [kq1[:,64h:64h+32]; kq2[:,64h:64h+32]],
    q_h likewise from rows 64h+32:64h+64.
  - K bias cancels in softmax (adds a per-query-row constant); dropped.
  - final BN + w_scale folded into out_w/out_b.
  - softmax without max-subtraction (max |score| ~ 67.5 < 88, fp32 safe).
  - softmax denominator via ones-augmented V (row 64 of PV output).

v3 structure (single software-pipelined stream):
  - All weights land in 2 merged DMAs; inputs in 3 merged DMAs per
    512-column tile (descriptor-gen on HWDGE costs ~650ns per DMA, so
    DMA count matters as much as bytes).
  - Attention groups for the j=0 query tile execute inside the
    DMA/projection window so PE never idles; S matmuls run LOOK=3
    groups ahead of their PV consumers so exp latency is hidden.
  - softmax exp split across three engines: true Exp on Act, and a
    Schraudolph fast-exp (int32(x*A+B) bitcast to f32, one
    tensor_scalar) on Pool and DVE for a subset of groups.  End-to-end
    rel err stays < 2e-3, inside the 2e-2 gate.
  - K scatter on Pool, Q/V bias + Vtok scatter on DVE; drain chains
    (reciprocal/broadcast/normalize) and the output projection are
    emitted a few stream steps late so PE never waits on them.
"""

import numpy as np
from contextlib import ExitStack

import concourse.bass as bass
import concourse.bacc as bacc
import concourse.mybir as mybir
import concourse.tile as tile
from concourse.bass_utils import run_bass_kernel_spmd
from concourse.masks import make_identity

F32 = mybir.dt.float32
F32R = mybir.dt.float32r
I32 = mybir.dt.int32
AF = mybir.ActivationFunctionType
ALU = bass.mybir.AluOpType

C = 256          # channels (INC1 == INC2)
N = 4096         # tokens per batch (64*64)
NQ = 2048        # queries per core
NT = 512         # free-dim tile size

# Schraudolph fast-exp: exp(s*0.125) ~ bitcast(int32(s*FE_A + FE_B))
FE_A = 0.125 * (2 ** 23) / float(np.log(2.0))
FE_B = 127.0 * (2 ** 23) - 486411.0

# exp engine per group: stream of 128 groups; first 32 (j=0, inside the
# DMA window where Pool runs the K scatters) -> Act; rest cycle this.
_POST_PATTERN = ['act', 'pool', 'act', 'dve', 'act', 'pool', 'act', 'pool']


def _exp_engine(i, ng):
    if i >= ng - 6:
        # stream tail: no S matmuls left to hide exp latency; use the two
        # lowest-latency engines alternately
        return 'act' if i % 2 else 'dve'
    if i < 32:
        return 'act'
    return _POST_PATTERN[(i - 32) % len(_POST_PATTERN)]


def build_bass():
    nc = bacc.Bacc("TRN2", target_bir_lowering=False, debug=False, num_devices=8)

    x1T = nc.dram_tensor("x1T", [C, N], F32R, kind="ExternalInput").ap()
    x2T = nc.dram_tensor("x2T", [C, N], F32R, kind="ExternalInput").ap()
    xT = nc.dram_tensor("xT", [C, N], F32R, kind="ExternalInput").ap()
    wkq_d = nc.dram_tensor("wkq", [128, 512], F32R, kind="ExternalInput").ap()
    wvo_d = nc.dram_tensor("wvo", [128, 512], F32R, kind="ExternalInput").ap()
    bcat_d = nc.dram_tensor("bcat", [128, 4], F32, kind="ExternalInput").ap()
    outT = nc.dram_tensor("outT", [C, NQ], F32, kind="ExternalOutput").ap()

    x1g = x1T.rearrange("(g p) c -> p g c", p=128)
    x2g = x2T.rearrange("(g p) c -> p g c", p=128)
    xg = xT.rearrange("(g p) c -> p g c", p=128)
    outg = outT.rearrange("(g p) c -> p g c", p=128)

    with ExitStack() as ctx:
        tc = ctx.enter_context(tile.TileContext(nc))
        const = ctx.enter_context(tc.tile_pool(name="const", bufs=1))
        pers = ctx.enter_context(tc.tile_pool(name="pers", bufs=1))

        # merged constants: wkq = [kq1 g0|g1, kq2 g0|g1]; wvo = [v g0|g1, wout]
        wkq = const.tile([128, 512], F32R, name="wkq")
        wvo = const.tile([128, 512], F32R, name="wvo")
        bcat = const.tile([128, 4], F32, name="bcat")
        ident = const.tile([128, 128], F32R, name="ident")
        ones1 = const.tile([1, 64], F32R, name="ones1")

        # wkq first so the t=0 projections can start ASAP; wvo/bcat are
        # issued inside the t=0 DMA block below.
        nc.sync.dma_start(out=wkq[:], in_=wkq_d[:])
        make_identity(nc, ident[:])
        nc.vector.memset(ones1[:], 1.0)

        w_kq1 = [wkq[:, 128 * g:128 * (g + 1)] for g in range(2)]
        w_kq2 = [wkq[:, 256 + 128 * g:256 + 128 * (g + 1)] for g in range(2)]
        w_v = [wvo[:, 128 * g:128 * (g + 1)] for g in range(2)]
        w_out = wvo[:, 256:512]
        b_q = bcat[:, 0:1]
        b_v = bcat[:, 1:2]
        b_out = [bcat[:, 2 + g:3 + g] for g in range(2)]

        # persistent SBUF
        KT = pers.tile([128, N], F32R, name="KT")      # rows k1a,k2a,k1b,k2b
        QT = pers.tile([128, NQ], F32R, name="QT")     # rows q1a,q2a,q1b,q2b
        Vtok = pers.tile([128, 32 * 130], F32R, name="Vtok")
        xsb = pers.tile([128, 2 * N], F32R, name="xsb")
        x1sb = pers.tile([128, 2 * N], F32R, name="x1sb")
        x2sb = pers.tile([128, 2 * N], F32R, name="x2sb")
        Ocat = pers.tile([128, NQ], F32R, name="Ocat")

        def gsl(g, cs):
            return slice(g * N + cs.start, g * N + cs.stop)

        # ones-columns of each Vtok m-block (softmax denominator rows)
        vtok3 = Vtok.rearrange("p (m c) -> p m c", c=130)
        nc.vector.memset(vtok3[:, :, 64:65], 1.0)
        nc.vector.memset(vtok3[:, :, 129:130], 1.0)

        # SBUF pools
        pvt = ctx.enter_context(tc.tile_pool(name="pvt", bufs=2))
        poolE = ctx.enter_context(tc.tile_pool(name="poolE", bufs=10))
        small = ctx.enter_context(tc.tile_pool(name="small", bufs=2))
        pout = ctx.enter_context(tc.tile_pool(name="pout", bufs=2))
        # PSUM pools live across both phases; sp is per-m-block so the
        # slot-reuse dependency (S(m) waits exp(m-4)) spans 4 m-blocks.
        poolS = ctx.enter_context(tc.tile_pool(name="poolS", bufs=4, space="PSUM"))
        poolO = ctx.enter_context(tc.tile_pool(name="poolO", bufs=2, space="PSUM"))

        # ---------------- stream definition ----------------
        # group i -> (h, j, g): j-major, g inner, heads interleaved.
        STREAM = [(h, j, g) for j in range(4) for g in range(16) for h in range(2)]
        NG = len(STREAM)  # 128
        LOOK = 3          # S/exp runs LOOK groups ahead of PV

        state = {
            'emitted': 0,       # S/exp emitted up to this stream index
            'pv': 0,            # PV emitted up to this stream index
            'step': 0,          # emission step counter (for deferred work)
            'pending': [],      # (due_step, fn) deferred emissions
            'sp': {},           # stream idx -> sp psum tile
            'et': {},           # stream idx -> et sbuf tile
            'op': {},           # (h, j) -> op psum tile
            'poolR': None,
            'poolC': None,
        }

        def flush(now):
            keep = []
            for due, fn in state['pending']:
                if due <= now:
                    fn()
                else:
                    keep.append((due, fn))
            state['pending'] = keep

        def defer(delta, fn):
            state['pending'].append((state['step'] + delta, fn))

        def emit_S_exp(i):
            h, j, g = STREAM[i]
            hs = slice(64 * h, 64 * (h + 1))
            qs = slice(j * NT, (j + 1) * NT)
            eng = _exp_engine(i, NG)
            ets = []
            for b2 in range(2):
                m = 2 * g + b2
                sp = poolS.tile([128, NT], F32, tag="sp", name=f"sp_{h}_{j}_{m}")
                nc.tensor.matmul(
                    sp[:],
                    KT[hs, m * 128:(m + 1) * 128],
                    QT[hs, qs],
                    start=True, stop=True)
                et = poolE.tile([128, NT], F32R, tag="et", name=f"et_{h}_{j}_{m}")
                if eng == 'act':
                    nc.scalar.activation(et[:], sp[:], AF.Exp, scale=0.125)
                else:
                    e = nc.gpsimd if eng == 'pool' else nc.vector
                    with nc.allow_low_precision(reason="schraudolph exp"):
                        e.tensor_scalar(et[:].bitcast(I32), sp[:], FE_A, FE_B,
                                        op0=ALU.mult, op1=ALU.add)
                ets.append(et)
            state['et'][i] = ets

        def emit_PV(i):
            h, j, g = STREAM[i]
            if g == 0:
                state['op'][(h, j)] = poolO.tile(
                    [65, NT], F32, tag="op", name=f"op_{h}_{j}")
            op = state['op'][(h, j)]
            ets = state['et'].pop(i)
            for b2 in range(2):
                m = 2 * g + b2
                nc.tensor.matmul(
                    op[:],
                    Vtok[:, m * 130 + 65 * h:m * 130 + 65 * h + 65],
                    ets[b2][:],
                    start=(m == 0),
                    stop=(m == 31))
            if g == 15:
                # reciprocal can start immediately (DVE); the PE/Pool/DVE
                # pieces of the drain are deferred so PE's queue never
                # blocks on them.
                hh, jj = h, j
                hsl = slice(64 * h, 64 * (h + 1))
                qsl = slice(j * NT, (j + 1) * NT)
                op_t = op
                rec = small.tile([1, NT], F32R, tag="rec", name=f"rec_{h}_{j}")
                with nc.allow_low_precision(reason="f32r is fp32 bits"):
                    nc.vector.reciprocal(rec[:], op_t[64:65, :])

                def drain():
                    rb = state['poolR'].tile([64, NT], F32, tag="rb",
                                             name=f"rb_{hh}_{jj}")
                    nc.tensor.matmul(rb[:], ones1[:], rec[:], start=True, stop=True)
                    nc.vector.tensor_mul(Ocat[hsl, qsl], op_t[0:64, :], rb[:])
                    del state['op'][(hh, jj)]

                defer(2, drain)
                if h == 1:
                    defer(3, lambda: emit_C0(jj))
                    defer(4, lambda: emit_C1(jj))

        def emit_C0(j):
            qs = slice(j * NT, (j + 1) * NT)
            pp = state['poolC'].tile([128, NT], F32, tag="pp", name=f"pp_{j}_0")
            nc.tensor.matmul(pp[:], w_out[:, 0:128], Ocat[:, qs],
                             start=True, stop=True)
            state[f'pp0_{j}'] = pp

        def emit_C1(j):
            qs = slice(j * NT, (j + 1) * NT)
            osb = pout.tile([128, 2 * NT], F32, tag="osb", name=f"osb_{j}")
            pp0 = state.pop(f'pp0_{j}')
            nc.vector.scalar_tensor_tensor(
                osb[:, 0:NT], pp0[:], b_out[0][:], xsb[:, gsl(0, qs)].bitcast(F32),
                op0=ALU.add, op1=ALU.add)
            nc.sync.dma_start(out=outg[:, 0, qs], in_=osb[:, 0:NT])
            pp = state['poolC'].tile([128, NT], F32, tag="pp", name=f"pp_{j}_1")
            nc.tensor.matmul(pp[:], w_out[:, 128:256], Ocat[:, qs],
                             start=True, stop=True)
            nc.vector.scalar_tensor_tensor(
                osb[:, NT:2 * NT], pp[:], b_out[1][:], xsb[:, gsl(1, qs)].bitcast(F32),
                op0=ALU.add, op1=ALU.add)
            nc.sync.dma_start(out=outg[:, 1, qs], in_=osb[:, NT:2 * NT])

        def pump(limit):
            """Emit S/exp up to stream index `limit`; PV trails by LOOK."""
            while state['emitted'] < limit:
                i = state['emitted']
                flush(state['step'])
                emit_S_exp(i)
                state['emitted'] = i + 1
                state['step'] += 1
                if i - LOOK >= 0:
                    emit_PV(i - LOOK)
                    state['pv'] = i - LOOK + 1

        def drain_stream():
            while state['pv'] < NG:
                flush(state['step'])
                emit_PV(state['pv'])
                state['pv'] += 1
                state['step'] += 1
            state['step'] += 100
            flush(state['step'])

        # ---------------- phase A window (+ j=0 attention) ----------------
        with ExitStack() as actx:
            poolA = actx.enter_context(tc.tile_pool(name="poolA", bufs=2, space="PSUM"))

            for t in range(8):
                cs = slice(t * NT, (t + 1) * NT)
                # merged input DMA for this tile (kq inputs first)
                nc.sync.dma_start(
                    out=x1sb.rearrange("p (g c) -> p g c", c=N)[:, :, cs],
                    in_=x1g[:, :, cs])
                nc.sync.dma_start(
                    out=x2sb.rearrange("p (g c) -> p g c", c=N)[:, :, cs],
                    in_=x2g[:, :, cs])
                if t == 0:
                    nc.sync.dma_start(out=wvo[:], in_=wvo_d[:])
                    nc.sync.dma_start(out=bcat[:], in_=bcat_d[:])
                nc.sync.dma_start(
                    out=xsb.rearrange("p (g c) -> p g c", c=N)[:, :, cs],
                    in_=xg[:, :, cs])

                # projections for tile t
                kq1p = poolA.tile([128, NT], F32, tag="mmA", name=f"kq1p_{t}")
                nc.tensor.matmul(kq1p[:], w_kq1[0], x1sb[:, gsl(0, cs)], start=True, stop=False)
                nc.tensor.matmul(kq1p[:], w_kq1[1], x1sb[:, gsl(1, cs)], start=False, stop=True)
                kq2p = poolA.tile([128, NT], F32, tag="mmA", name=f"kq2p_{t}")
                nc.tensor.matmul(kq2p[:], w_kq2[0], x2sb[:, gsl(0, cs)], start=True, stop=False)
                nc.tensor.matmul(kq2p[:], w_kq2[1], x2sb[:, gsl(1, cs)], start=False, stop=True)

                # K scatter on Pool (kq1p consumers first so its psum slot
                # frees early), Q bias on DVE
                nc.gpsimd.tensor_copy(KT[0:32, cs], kq1p[0:32, :])
                nc.gpsimd.tensor_copy(KT[64:96, cs], kq1p[64:96, :])
                nc.gpsimd.tensor_copy(KT[32:64, cs], kq2p[0:32, :])
                nc.gpsimd.tensor_copy(KT[96:128, cs], kq2p[64:96, :])
                if t < 4:  # query half
                    nc.vector.tensor_scalar(QT[0:32, cs], kq1p[32:64, :],
                                            b_q[0:32, :], None, op0=ALU.add)
                    nc.vector.tensor_scalar(QT[64:96, cs], kq1p[96:128, :],
                                            b_q[64:96, :], None, op0=ALU.add)
                    nc.vector.tensor_scalar(QT[32:64, cs], kq2p[32:64, :],
                                            b_q[32:64, :], None, op0=ALU.add)
                    nc.vector.tensor_scalar(QT[96:128, cs], kq2p[96:128, :],
                                            b_q[96:128, :], None, op0=ALU.add)

                # older groups keep PE busy while Pool/DVE drain the kq PSUMs
                pump(4 * t)

                vp = poolA.tile([128, NT], F32, tag="mmA", name=f"vp_{t}")
                nc.tensor.matmul(vp[:], w_v[0], xsb[:, gsl(0, cs)], start=True, stop=False)
                nc.tensor.matmul(vp[:], w_v[1], xsb[:, gsl(1, cs)], start=False, stop=True)
                VT = pvt.tile([128, NT], F32R, tag="VT", name=f"VT_{t}")
                nc.vector.tensor_scalar(VT[:], vp[:], b_v[:], None, op0=ALU.add)

                # newest groups (need this tile's K) before the transposes
                pump(min(4 * t + 4, 32))

                # transpose V for PV matmuls: one psum tile, 4 blocks
                tpw = poolA.tile([128, NT], F32R, tag="mmA", name=f"tpw_{t}")
                for s in range(4):
                    nc.tensor.transpose(tpw[:, s * 128:(s + 1) * 128],
                                        VT[:, s * 128:(s + 1) * 128], ident[:])
                # scatter into Vtok (cols 0:64 = v1, 65:129 = v2) in 2 copies
                tp4 = tpw.rearrange("p (s c) -> p s c", c=128)
                vt4 = vtok3[:, 4 * t:4 * t + 4, :]
                nc.vector.tensor_copy(vt4[:, :, 0:64], tp4[:, :, 0:64])
                nc.vector.tensor_copy(vt4[:, :, 65:129], tp4[:, :, 64:128])

        # ---------------- post-window: rest of attention + output ----------
        with ExitStack() as bctx:
            state['poolR'] = bctx.enter_context(
                tc.tile_pool(name="poolR", bufs=1, space="PSUM"))
            state['poolC'] = bctx.enter_context(
                tc.tile_pool(name="poolC", bufs=1, space="PSUM"))
            pump(NG)
            drain_stream()

    nc.compile()
    return nc


_NC = None


def _get_nc():
    global _NC
    if _NC is None:
        _NC = build_bass()
    return _NC


def kernel(**inputs):
    out, _ = _run(inputs, trace=False)
    return out


def _run(inputs, trace=False):
    eps = 1e-5
    f32 = np.float32
    inp = {k: np.asarray(v, dtype=np.float32) for k, v in inputs.items()}

    s1 = inp['bn1_g'] / np.sqrt(inp['bn1_v'] + eps)
    t1 = inp['bn1_b'] - inp['bn1_m'] * s1
    s2 = inp['bn2_g'] / np.sqrt(inp['bn2_v'] + eps)
    t2 = inp['bn2_b'] - inp['bn2_m'] * s2
    W1 = inp['kq1_w'] * s1[None, :]
    b1 = inp['kq1_b'] + inp['kq1_w'] @ t1
    W2 = inp['kq2_w'] * s2[None, :]
    b2 = inp['kq2_b'] + inp['kq2_w'] @ t2
    sl = inp['bnl_g'] / np.sqrt(inp['bnl_v'] + eps)
    tl = inp['bnl_b'] - inp['bnl_m'] * sl
    ws = inp['w_scale'][0]
    Wout = (ws * sl)[:, None] * inp['out_w']
    bout_f = ws * (sl * inp['out_b'] + tl)

    # merged weights: wkq = [kq1 g0|g1, kq2 g0|g1]; wvo = [v g0|g1, wout]
    wkq = np.concatenate([
        W1.T[0:128], W1.T[128:256],
        W2.T[0:128], W2.T[128:256],
    ], axis=1).astype(f32)
    wvo = np.concatenate([
        inp['v_w'].T[0:128], inp['v_w'].T[128:256],
        Wout.T,
    ], axis=1).astype(f32)
    bq = np.concatenate([b1[32:64], b2[32:64], b1[96:128], b2[96:128]])
    bcat = np.stack([bq, inp['v_b'], bout_f[0:128], bout_f[128:256]],
                    axis=1).astype(f32)

    shared = dict(wkq=np.ascontiguousarray(wkq),
                  wvo=np.ascontiguousarray(wvo),
                  bcat=np.ascontiguousarray(bcat))

    in_maps = []
    for b in range(4):
        x1Tb = inp['x1'][b].reshape(C, N)
        x2Tb = inp['x2'][b].reshape(C, N)
        xTb = inp['x'][b].reshape(C, N)
        for qh in range(2):
            if qh == 0:
                m = dict(x1T=np.ascontiguousarray(x1Tb),
                         x2T=np.ascontiguousarray(x2Tb),
                         xT=np.ascontiguousarray(xTb))
            else:
                m = dict(x1T=np.roll(x1Tb, -NQ, axis=1),
                         x2T=np.roll(x2Tb, -NQ, axis=1),
                         xT=np.roll(xTb, -NQ, axis=1))
            m.update(shared)
            in_maps.append(m)

    nc = _get_nc()
    res = run_bass_kernel_spmd(nc, in_maps, list(range(8)), trace=trace)

    out = np.empty((4, C, 64, 64), dtype=f32)
    for b in range(4):
        full = np.empty((C, N), dtype=f32)
        full[:, 0:NQ] = res.results[2 * b]["outT"]
        full[:, NQ:N] = res.results[2 * b + 1]["outT"]
        out[b] = full.reshape(C, 64, 64)
    return out, res


# revision 21
# speedup vs baseline: 1.4609x; 1.0292x over previous
"""Cross_Atten_Lite_split Trainium2 Bass kernel (v3 — pipelined).

Sharding: 8 cores = (batch b in 0..3) x (query-half qh in 0..1).
Each core computes both attention heads for 2048 queries x 4096 keys of
its batch. No collectives. Math rewrites (validated vs reference):
  - eval-mode BN on x1/x2 folded into kq1_w/kq2_w (+bias).
  - channel_shuffle is a permutation of the shared q/k contraction axis
    -> eliminated;  k_h = [kq1[:,64h:64h+32]; kq2[:,64h:64h+32]],
    q_h likewise from rows 64h+32:64h+64.
  - K bias cancels in softmax (adds a per-query-row constant); dropped.
  - final BN + w_scale folded into out_w/out_b.
  - softmax without max-subtraction (max |score| ~ 67.5 < 88, fp32 safe).
  - softmax denominator via ones-augmented V (row 64 of PV output).

v3 structure (single software-pipelined stream):
  - All weights land in 2 merged DMAs; inputs in 3 merged DMAs per
    512-column tile (descriptor-gen on HWDGE costs ~650ns per DMA, so
    DMA count matters as much as bytes).
  - Attention groups for the j=0 query tile execute inside the
    DMA/projection window so PE never idles; S matmuls run LOOK=3
    groups ahead of their PV consumers so exp latency is hidden.
  - softmax exp split across three engines: true Exp on Act, and a
    Schraudolph fast-exp (int32(x*A+B) bitcast to f32, one
    tensor_scalar) on Pool and DVE for a subset of groups.  End-to-end
    rel err stays < 2e-3, inside the 2e-2 gate.
  - K scatter on Pool, Q/V bias + Vtok scatter on DVE; drain chains
    (reciprocal/broadcast/normalize) and the output projection are
    emitted a few stream steps late so PE never waits on them.
"""

import numpy as np
from contextlib import ExitStack

import concourse.bass as bass
import concourse.bacc as bacc
import concourse.mybir as mybir
import concourse.tile as tile
from concourse.bass_utils import run_bass_kernel_spmd
from concourse.masks import make_identity

F32 = mybir.dt.float32
F32R = mybir.dt.float32r
I32 = mybir.dt.int32
AF = mybir.ActivationFunctionType
ALU = bass.mybir.AluOpType

C = 256          # channels (INC1 == INC2)
N = 4096         # tokens per batch (64*64)
NQ = 2048        # queries per core
NT = 512         # free-dim tile size

# Schraudolph fast-exp: exp(s*0.125) ~ bitcast(int32(s*FE_A + FE_B))
FE_A = 0.125 * (2 ** 23) / float(np.log(2.0))
FE_B = 127.0 * (2 ** 23) - 486411.0

# exp engine per group: stream of 128 groups; first 32 (j=0, inside the
# DMA window where Pool runs the K scatters) -> Act; rest cycle this.
_POST_PATTERN = ['act', 'pool', 'act', 'dve', 'act', 'pool', 'act', 'dve']


def _exp_engine(i, ng):
    if i >= ng - 3:
        # stream tail: the last PVs have no S matmuls covering them; put
        # each group's exp on a different engine so they run concurrently
        return ['pool', 'dve', 'act'][i - (ng - 3)]
    if i < 32:
        return 'act'
    eng = _POST_PATTERN[(i - 32) % len(_POST_PATTERN)]
    if eng == 'dve' and (i % 32) < 6:
        # j-boundary: DVE is busy with drain/normalize chains
        eng = 'act'
    return eng


def build_bass():
    nc = bacc.Bacc("TRN2", target_bir_lowering=False, debug=False, num_devices=8)

    x1T = nc.dram_tensor("x1T", [C, N], F32R, kind="ExternalInput").ap()
    x2T = nc.dram_tensor("x2T", [C, N], F32R, kind="ExternalInput").ap()
    xT = nc.dram_tensor("xT", [C, N], F32R, kind="ExternalInput").ap()
    wkq_d = nc.dram_tensor("wkq", [128, 512], F32R, kind="ExternalInput").ap()
    wvo_d = nc.dram_tensor("wvo", [128, 512], F32R, kind="ExternalInput").ap()
    bcat_d = nc.dram_tensor("bcat", [128, 4], F32, kind="ExternalInput").ap()
    outT = nc.dram_tensor("outT", [C, NQ], F32, kind="ExternalOutput").ap()

    x1g = x1T.rearrange("(g p) c -> p g c", p=128)
    x2g = x2T.rearrange("(g p) c -> p g c", p=128)
    xg = xT.rearrange("(g p) c -> p g c", p=128)
    outg = outT.rearrange("(g p) c -> p g c", p=128)

    with ExitStack() as ctx:
        tc = ctx.enter_context(tile.TileContext(nc))
        const = ctx.enter_context(tc.tile_pool(name="const", bufs=1))
        pers = ctx.enter_context(tc.tile_pool(name="pers", bufs=1))

        # merged constants: wkq = [kq1 g0|g1, kq2 g0|g1]; wvo = [v g0|g1, wout]
        wkq = const.tile([128, 512], F32R, name="wkq")
        wvo = const.tile([128, 512], F32R, name="wvo")
        bcat = const.tile([128, 4], F32, name="bcat")
        ident = const.tile([128, 128], F32R, name="ident")
        ones1 = const.tile([1, 64], F32R, name="ones1")

        # wkq first so the t=0 projections can start ASAP; wvo/bcat are
        # issued inside the t=0 DMA block below.
        nc.sync.dma_start(out=wkq[:], in_=wkq_d[:])
        make_identity(nc, ident[:])
        nc.vector.memset(ones1[:], 1.0)

        w_kq1 = [wkq[:, 128 * g:128 * (g + 1)] for g in range(2)]
        w_kq2 = [wkq[:, 256 + 128 * g:256 + 128 * (g + 1)] for g in range(2)]
        w_v = [wvo[:, 128 * g:128 * (g + 1)] for g in range(2)]
        w_out = wvo[:, 256:512]
        b_q = bcat[:, 0:1]
        b_v = bcat[:, 1:2]
        b_out = [bcat[:, 2 + g:3 + g] for g in range(2)]

        # persistent SBUF
        KT = pers.tile([128, N], F32R, name="KT")      # rows k1a,k2a,k1b,k2b
        QT = pers.tile([128, NQ], F32R, name="QT")     # rows q1a,q2a,q1b,q2b
        Vtok = pers.tile([128, 32 * 130], F32R, name="Vtok")
        xsb = pers.tile([128, 2 * N], F32R, name="xsb")
        x1sb = pers.tile([128, 2 * N], F32R, name="x1sb")
        x2sb = pers.tile([128, 2 * N], F32R, name="x2sb")
        Ocat = pers.tile([128, NQ], F32R, name="Ocat")

        def gsl(g, cs):
            return slice(g * N + cs.start, g * N + cs.stop)

        # ones-columns of each Vtok m-block (softmax denominator rows)
        vtok3 = Vtok.rearrange("p (m c) -> p m c", c=130)
        nc.vector.memset(vtok3[:, :, 64:65], 1.0)
        nc.vector.memset(vtok3[:, :, 129:130], 1.0)

        # SBUF pools
        pvt = ctx.enter_context(tc.tile_pool(name="pvt", bufs=2))
        poolE = ctx.enter_context(tc.tile_pool(name="poolE", bufs=10))
        small = ctx.enter_context(tc.tile_pool(name="small", bufs=2))
        pout = ctx.enter_context(tc.tile_pool(name="pout", bufs=2))
        # PSUM pools live across both phases; sp is per-m-block so the
        # slot-reuse dependency (S(m) waits exp(m-4)) spans 4 m-blocks.
        poolS = ctx.enter_context(tc.tile_pool(name="poolS", bufs=4, space="PSUM"))
        poolO = ctx.enter_context(tc.tile_pool(name="poolO", bufs=2, space="PSUM"))

        # ---------------- stream definition ----------------
        # group i -> (h, j, g): j-major, g inner, heads interleaved.
        STREAM = [(h, j, g) for j in range(4) for g in range(16) for h in range(2)]
        NG = len(STREAM)  # 128
        LOOK = 3          # S/exp runs LOOK groups ahead of PV

        state = {
            'emitted': 0,       # S/exp emitted up to this stream index
            'pv': 0,            # PV emitted up to this stream index
            'step': 0,          # emission step counter (for deferred work)
            'pending': [],      # (due_step, fn) deferred emissions
            'sp': {},           # stream idx -> sp psum tile
            'et': {},           # stream idx -> et sbuf tile
            'op': {},           # (h, j) -> op psum tile
            'poolR': None,
            'poolC': None,
        }

        def flush(now):
            keep = []
            for due, fn in state['pending']:
                if due <= now:
                    fn()
                else:
                    keep.append((due, fn))
            state['pending'] = keep

        def defer(delta, fn):
            state['pending'].append((state['step'] + delta, fn))

        def emit_S_exp(i):
            h, j, g = STREAM[i]
            hs = slice(64 * h, 64 * (h + 1))
            qs = slice(j * NT, (j + 1) * NT)
            eng = _exp_engine(i, NG)
            ets = []
            for b2 in range(2):
                m = 2 * g + b2
                sp = poolS.tile([128, NT], F32, tag="sp", name=f"sp_{h}_{j}_{m}")
                nc.tensor.matmul(
                    sp[:],
                    KT[hs, m * 128:(m + 1) * 128],
                    QT[hs, qs],
                    start=True, stop=True)
                et = poolE.tile([128, NT], F32R, tag="et", name=f"et_{h}_{j}_{m}")
                if eng == 'act':
                    nc.scalar.activation(et[:], sp[:], AF.Exp, scale=0.125)
                else:
                    e = nc.gpsimd if eng == 'pool' else nc.vector
                    with nc.allow_low_precision(reason="schraudolph exp"):
                        e.tensor_scalar(et[:].bitcast(I32), sp[:], FE_A, FE_B,
                                        op0=ALU.mult, op1=ALU.add)
                ets.append(et)
            state['et'][i] = ets

        def emit_PV(i):
            h, j, g = STREAM[i]
            if g == 0:
                state['op'][(h, j)] = poolO.tile(
                    [65, NT], F32, tag="op", name=f"op_{h}_{j}")
            op = state['op'][(h, j)]
            ets = state['et'].pop(i)
            for b2 in range(2):
                m = 2 * g + b2
                nc.tensor.matmul(
                    op[:],
                    Vtok[:, m * 130 + 65 * h:m * 130 + 65 * h + 65],
                    ets[b2][:],
                    start=(m == 0),
                    stop=(m == 31))
            if g == 15:
                # reciprocal can start immediately (DVE); the PE/Pool/DVE
                # pieces of the drain are deferred so PE's queue never
                # blocks on them.
                hh, jj = h, j
                hsl = slice(64 * h, 64 * (h + 1))
                qsl = slice(j * NT, (j + 1) * NT)
                op_t = op
                rec = small.tile([1, NT], F32R, tag="rec", name=f"rec_{h}_{j}")
                with nc.allow_low_precision(reason="f32r is fp32 bits"):
                    nc.vector.reciprocal(rec[:], op_t[64:65, :])

                def drain():
                    rb = state['poolR'].tile([64, NT], F32, tag="rb",
                                             name=f"rb_{hh}_{jj}")
                    nc.tensor.matmul(rb[:], ones1[:], rec[:], start=True, stop=True)
                    nc.vector.tensor_mul(Ocat[hsl, qsl], op_t[0:64, :], rb[:])
                    del state['op'][(hh, jj)]

                defer(2 + h, drain)
                if h == 1:
                    defer(5, lambda: emit_C0(jj))
                    defer(7, lambda: emit_C1(jj))

        def emit_C0(j):
            qs = slice(j * NT, (j + 1) * NT)
            pp = state['poolC'].tile([128, NT], F32, tag="pp", name=f"pp_{j}_0")
            nc.tensor.matmul(pp[:], w_out[:, 0:128], Ocat[:, qs],
                             start=True, stop=True)
            state[f'pp0_{j}'] = pp

        def emit_C1(j):
            qs = slice(j * NT, (j + 1) * NT)
            osb = pout.tile([128, 2 * NT], F32, tag="osb", name=f"osb_{j}")
            pp0 = state.pop(f'pp0_{j}')
            nc.vector.scalar_tensor_tensor(
                osb[:, 0:NT], pp0[:], b_out[0][:], xsb[:, gsl(0, qs)].bitcast(F32),
                op0=ALU.add, op1=ALU.add)
            nc.sync.dma_start(out=outg[:, 0, qs], in_=osb[:, 0:NT])
            pp = state['poolC'].tile([128, NT], F32, tag="pp", name=f"pp_{j}_1")
            nc.tensor.matmul(pp[:], w_out[:, 128:256], Ocat[:, qs],
                             start=True, stop=True)
            nc.vector.scalar_tensor_tensor(
                osb[:, NT:2 * NT], pp[:], b_out[1][:], xsb[:, gsl(1, qs)].bitcast(F32),
                op0=ALU.add, op1=ALU.add)
            nc.sync.dma_start(out=outg[:, 1, qs], in_=osb[:, NT:2 * NT])

        def pump(limit):
            """Emit S/exp up to stream index `limit`; PV trails by LOOK."""
            while state['emitted'] < limit:
                i = state['emitted']
                flush(state['step'])
                emit_S_exp(i)
                state['emitted'] = i + 1
                state['step'] += 1
                if i - LOOK >= 0:
                    emit_PV(i - LOOK)
                    state['pv'] = i - LOOK + 1

        def drain_stream():
            while state['pv'] < NG:
                flush(state['step'])
                emit_PV(state['pv'])
                state['pv'] += 1
                state['step'] += 1
            state['step'] += 100
            flush(state['step'])

        # ---------------- phase A window (+ j=0 attention) ----------------
        with ExitStack() as actx:
            poolA = actx.enter_context(tc.tile_pool(name="poolA", bufs=2, space="PSUM"))

            for t in range(8):
                cs = slice(t * NT, (t + 1) * NT)
                # merged input DMA for this tile (kq inputs first)
                nc.sync.dma_start(
                    out=x1sb.rearrange("p (g c) -> p g c", c=N)[:, :, cs],
                    in_=x1g[:, :, cs])
                nc.sync.dma_start(
                    out=x2sb.rearrange("p (g c) -> p g c", c=N)[:, :, cs],
                    in_=x2g[:, :, cs])
                if t == 0:
                    nc.sync.dma_start(out=wvo[:], in_=wvo_d[:])
                    nc.sync.dma_start(out=bcat[:], in_=bcat_d[:])
                nc.sync.dma_start(
                    out=xsb.rearrange("p (g c) -> p g c", c=N)[:, :, cs],
                    in_=xg[:, :, cs])

                # projections for tile t
                kq1p = poolA.tile([128, NT], F32, tag="mmA", name=f"kq1p_{t}")
                nc.tensor.matmul(kq1p[:], w_kq1[0], x1sb[:, gsl(0, cs)], start=True, stop=False)
                nc.tensor.matmul(kq1p[:], w_kq1[1], x1sb[:, gsl(1, cs)], start=False, stop=True)
                kq2p = poolA.tile([128, NT], F32, tag="mmA", name=f"kq2p_{t}")
                nc.tensor.matmul(kq2p[:], w_kq2[0], x2sb[:, gsl(0, cs)], start=True, stop=False)
                nc.tensor.matmul(kq2p[:], w_kq2[1], x2sb[:, gsl(1, cs)], start=False, stop=True)

                # K scatter: head-0 rows on Pool, head-1 rows on DVE (both
                # straight from PSUM, kq1p consumers first so its psum slot
                # frees early); Q bias on DVE
                nc.gpsimd.tensor_copy(KT[0:32, cs], kq1p[0:32, :])
                nc.vector.tensor_copy(KT[64:96, cs], kq1p[64:96, :])
                nc.gpsimd.tensor_copy(KT[32:64, cs], kq2p[0:32, :])
                nc.vector.tensor_copy(KT[96:128, cs], kq2p[64:96, :])
                if t < 4:  # query half
                    nc.gpsimd.tensor_scalar(QT[0:32, cs], kq1p[32:64, :],
                                            b_q[0:32, :], None, op0=ALU.add)
                    nc.vector.tensor_scalar(QT[64:96, cs], kq1p[96:128, :],
                                            b_q[64:96, :], None, op0=ALU.add)
                    nc.gpsimd.tensor_scalar(QT[32:64, cs], kq2p[32:64, :],
                                            b_q[32:64, :], None, op0=ALU.add)
                    nc.vector.tensor_scalar(QT[96:128, cs], kq2p[96:128, :],
                                            b_q[96:128, :], None, op0=ALU.add)

                # older groups keep PE busy while Pool/DVE drain the kq PSUMs
                pump(4 * t)

                vp = poolA.tile([128, NT], F32, tag="mmA", name=f"vp_{t}")
                nc.tensor.matmul(vp[:], w_v[0], xsb[:, gsl(0, cs)], start=True, stop=False)
                nc.tensor.matmul(vp[:], w_v[1], xsb[:, gsl(1, cs)], start=False, stop=True)
                VT = pvt.tile([128, NT], F32R, tag="VT", name=f"VT_{t}")
                nc.vector.tensor_scalar(VT[:], vp[:], b_v[:], None, op0=ALU.add)

                # newest groups (need this tile's K) before the transposes
                pump(min(4 * t + 4, 32))

                # transpose V for PV matmuls: one psum tile, 4 blocks
                tpw = poolA.tile([128, NT], F32R, tag="mmA", name=f"tpw_{t}")
                for s in range(4):
                    nc.tensor.transpose(tpw[:, s * 128:(s + 1) * 128],
                                        VT[:, s * 128:(s + 1) * 128], ident[:])
                # scatter into Vtok (cols 0:64 = v1, 65:129 = v2) in 2 copies
                tp4 = tpw.rearrange("p (s c) -> p s c", c=128)
                vt4 = vtok3[:, 4 * t:4 * t + 4, :]
                nc.vector.tensor_copy(vt4[:, :, 0:64], tp4[:, :, 0:64])
                nc.vector.tensor_copy(vt4[:, :, 65:129], tp4[:, :, 64:128])

        # ---------------- post-window: rest of attention + output ----------
        with ExitStack() as bctx:
            state['poolR'] = bctx.enter_context(
                tc.tile_pool(name="poolR", bufs=1, space="PSUM"))
            state['poolC'] = bctx.enter_context(
                tc.tile_pool(name="poolC", bufs=1, space="PSUM"))
            pump(NG)
            drain_stream()

    nc.compile()
    return nc


_NC = None


def _get_nc():
    global _NC
    if _NC is None:
        _NC = build_bass()
    return _NC


def kernel(**inputs):
    out, _ = _run(inputs, trace=False)
    return out


def _run(inputs, trace=False):
    eps = 1e-5
    f32 = np.float32
    inp = {k: np.asarray(v, dtype=np.float32) for k, v in inputs.items()}

    s1 = inp['bn1_g'] / np.sqrt(inp['bn1_v'] + eps)
    t1 = inp['bn1_b'] - inp['bn1_m'] * s1
    s2 = inp['bn2_g'] / np.sqrt(inp['bn2_v'] + eps)
    t2 = inp['bn2_b'] - inp['bn2_m'] * s2
    W1 = inp['kq1_w'] * s1[None, :]
    b1 = inp['kq1_b'] + inp['kq1_w'] @ t1
    W2 = inp['kq2_w'] * s2[None, :]
    b2 = inp['kq2_b'] + inp['kq2_w'] @ t2
    sl = inp['bnl_g'] / np.sqrt(inp['bnl_v'] + eps)
    tl = inp['bnl_b'] - inp['bnl_m'] * sl
    ws = inp['w_scale'][0]
    Wout = (ws * sl)[:, None] * inp['out_w']
    bout_f = ws * (sl * inp['out_b'] + tl)

    # merged weights: wkq = [kq1 g0|g1, kq2 g0|g1]; wvo = [v g0|g1, wout]
    wkq = np.concatenate([
        W1.T[0:128], W1.T[128:256],
        W2.T[0:128], W2.T[128:256],
    ], axis=1).astype(f32)
    wvo = np.concatenate([
        inp['v_w'].T[0:128], inp['v_w'].T[128:256],
        Wout.T,
    ], axis=1).astype(f32)
    bq = np.concatenate([b1[32:64], b2[32:64], b1[96:128], b2[96:128]])
    bcat = np.stack([bq, inp['v_b'], bout_f[0:128], bout_f[128:256]],
                    axis=1).astype(f32)

    shared = dict(wkq=np.ascontiguousarray(wkq),
                  wvo=np.ascontiguousarray(wvo),
                  bcat=np.ascontiguousarray(bcat))

    in_maps = []
    for b in range(4):
        x1Tb = inp['x1'][b].reshape(C, N)
        x2Tb = inp['x2'][b].reshape(C, N)
        xTb = inp['x'][b].reshape(C, N)
        for qh in range(2):
            if qh == 0:
                m = dict(x1T=np.ascontiguousarray(x1Tb),
                         x2T=np.ascontiguousarray(x2Tb),
                         xT=np.ascontiguousarray(xTb))
            else:
                m = dict(x1T=np.roll(x1Tb, -NQ, axis=1),
                         x2T=np.roll(x2Tb, -NQ, axis=1),
                         xT=np.roll(xTb, -NQ, axis=1))
            m.update(shared)
            in_maps.append(m)

    nc = _get_nc()
    res = run_bass_kernel_spmd(nc, in_maps, list(range(8)), trace=trace)

    out = np.empty((4, C, 64, 64), dtype=f32)
    for b in range(4):
        full = np.empty((C, N), dtype=f32)
        full[:, 0:NQ] = res.results[2 * b]["outT"]
        full[:, NQ:N] = res.results[2 * b + 1]["outT"]
        out[b] = full.reshape(C, 64, 64)
    return out, res
